# revision 47
# baseline (speedup 1.0000x reference)
"""Trainium2 Bass kernel for nn_BilinearDense.

Math:
  W = (z @ W_kernel + W_bias).reshape(B, OD, XD)      # per-sample matrix
  b = z @ b_kernel + b_bias                            # per-sample bias
  out[b,o] = sum_i W[b,o,i] x[b,i] + b[b,o]
           = sum_{k,i} z[b,k] x[b,i] W_kernel[k, o*XD+i]  (+ bias terms)

Strategy (8 NeuronCores, batch-sharded, 512 samples/core):
  One long PE accumulation per core:
     outT[o, b] = sum_{ki} Wt[ki, o] * PT[ki, b]
  with Wt[(k,i), o] = W_kernel[k, o*XD+i] and PT[(k,i), b] = z[b,k]*x[b,i]
  built on-chip by the Vector engine (x comes from a 0-stride broadcast AP
  over one resident [XD, BS] tile; z is pre-broadcast in HBM as zrep).

  Mixed precision: 9 of 33 k-groups run as fp8 e4m3 DoubleRow matmuls
  (2 k's contracted per PE pass = 2x FLOP rate, HW-verified); their PT
  is converted bf16 -> e4m3 by the otherwise-idle Activation engine
  (DVE fp8 writes run at 1x and would starve the PE).  Three startup
  groups use e3m4 W (mixed-dtype matmul at full bf16 rate, half DMA).
  Scales: x is pre-divided by 32, bf16/e3m4 W slabs carry x32, fp8 W is
  unscaled with the x32 applied in the Act conversion -- so PSUM holds
  final-scale values and drains with plain copies.  fp8 groups are
  spread every 3rd group: fp8 blocks are locally Act/DVE-bound and must
  interleave with PE-bound bf16 blocks.  Short warm-up matmuls on a
  memset tile ramp the PE clock during the DMA-bound startup.
  Measured rel-err 1.752e-2 (< 2e-2 gate), HW ~221-223us vs 255.6us
  bf16 baseline.
"""

import numpy as np
import ml_dtypes

B, XD, ZD, OD = 4096, 256, 256, 256
NCORES = 8
BS = B // NCORES  # batch shard per core
KG = 8            # k's per group (z-replica slab granularity)
NG = ZD // KG     # 32 groups
P = 128

W_SCALE = 32.0
OUT_DESCALE = 1.0 / W_SCALE
# k-group sizes: two small startup groups (faster pipeline fill), then 8s
KS = [4, 4] + [8] * 31
NG = len(KS)
FP8_GROUPS = (5, 8, 11, 14, 17, 20, 23, 26, 29)  # e4m3 DoubleRow groups
E3_GROUPS = (0, 1, 2)                        # e3m4 W (halves startup W DMA)

BF = ml_dtypes.bfloat16
E4 = ml_dtypes.float8_e4m3
E3 = ml_dtypes.float8_e3m4

_prog_cache = {}


def _build_program(skip_bias=False):
    key = ("nc", skip_bias)
    if key in _prog_cache:
        return _prog_cache[key], "outT"

    import concourse.bass as bass
    import concourse.tile as tile
    from concourse import bacc, mybir

    bf16 = mybir.dt.bfloat16
    f8e4 = mybir.dt.float8e4
    f8e3 = mybir.dt.float8e3
    f32 = mybir.dt.float32

    nc = bacc.Bacc(
        "TRN2", target_bir_lowering=False, debug=False, num_devices=NCORES
    )

    kb = sum(kg for g, kg in enumerate(KS)
             if g not in FP8_GROUPS and g not in E3_GROUPS)
    k8 = sum(KS[g] for g in FP8_GROUPS)
    k3 = sum(KS[g] for g in E3_GROUPS)
    d_xt = nc.dram_tensor("xt", [XD, BS], bf16, kind="ExternalInput").ap()
    d_zrep = nc.dram_tensor("zrep", [ZD * P * BS], bf16, kind="ExternalInput").ap()
    d_wtb = nc.dram_tensor("wtb", [kb * 2 * P * OD], bf16, kind="ExternalInput").ap()
    d_wt8 = nc.dram_tensor("wt8", [k8 * 2 * P * OD], f8e4, kind="ExternalInput").ap()
    d_wt3 = nc.dram_tensor("wt3", [k3 * 2 * P * OD], f8e3, kind="ExternalInput").ap()
    d_zt = nc.dram_tensor("zt", [P, 2 * BS], bf16, kind="ExternalInput").ap()
    d_bk = nc.dram_tensor("bk", [ZD, OD], bf16, kind="ExternalInput").ap()
    d_wbt = nc.dram_tensor("wbt", [XD, OD], bf16, kind="ExternalInput").ap()
    d_bb = nc.dram_tensor("bb", [1, OD], bf16, kind="ExternalInput").ap()
    d_ones = nc.dram_tensor("ones", [1, BS], bf16, kind="ExternalInput").ap()
    d_out = nc.dram_tensor("outT", [OD, BS], f32, kind="ExternalOutput").ap()

    # slab offsets (separate streams per W dtype)
    wt_off = {}
    zr_off = {}
    offb = off8 = off3 = zoff = 0
    for _g, _kg in enumerate(KS):
        zr_off[_g] = zoff
        zoff += P * _kg * BS
        for _ih in range(2):
            if _g in FP8_GROUPS:
                wt_off[(_g, _ih)] = off8
                off8 += P * _kg * OD
            elif _g in E3_GROUPS:
                wt_off[(_g, _ih)] = off3
                off3 += P * _kg * OD
            else:
                wt_off[(_g, _ih)] = offb
                offb += P * _kg * OD

    def zs_load(engine, zs, g, kg, part=None):
        o = zr_off[g]
        src = d_zrep[o : o + P * kg * BS].rearrange("(p f) -> p f", p=P)
        if part is None:
            engine.dma_start(zs[:], src)
        else:
            i, n = part
            F = kg * BS // n
            engine.dma_start(zs[:, i * F : (i + 1) * F], src[:, i * F : (i + 1) * F])

    def ws_load(engine, ws, g, kg, ih, part=None):
        o = wt_off[(g, ih)]
        d = (
            d_wt8
            if g in FP8_GROUPS
            else (d_wt3 if g in E3_GROUPS else d_wtb)
        )
        src = d[o : o + P * kg * OD].rearrange("(p f) -> p f", p=P)
        if part is None:
            engine.dma_start(ws[:], src)
        else:
            i, n = part
            F = kg * OD // n
            engine.dma_start(ws[:, i * F : (i + 1) * F], src[:, i * F : (i + 1) * F])

    with tile.TileContext(nc) as tc:
        with (
            tc.tile_pool(name="const", bufs=1) as cpool,
            tc.tile_pool(name="zslab", bufs=5) as zpool,
            tc.tile_pool(name="wslab", bufs=5) as wpool,
            tc.tile_pool(name="pt", bufs=4) as ptpool,
            tc.tile_pool(name="outp", bufs=1) as opool,
            tc.tile_pool(name="psum", bufs=1, space="PSUM") as psum,
        ):
            # ---- initial loads, most-critical first ----
            xt = [
                cpool.tile([P, BS], bf16, tag=f"xt{ih}", name=f"xt{ih}")
                for ih in range(2)
            ]
            kg0, kg1 = KS[0], KS[1]
            zs0 = zpool.tile([P, kg0 * BS], bf16, tag="zs", name="zs0")
            ws00 = wpool.tile([P, kg0 * OD], f8e3, tag="ws3", name="ws00")
            ws01 = wpool.tile([P, kg0 * OD], f8e3, tag="ws3", name="ws01")

            nc.sync.dma_start(xt[0][:], d_xt[0:P, :])
            zs_load(nc.sync, zs0, 0, kg0, part=(0, 2))
            ws_load(nc.sync, ws00, 0, kg0, 0)
            zs_load(nc.sync, zs0, 0, kg0, part=(1, 2))
            nc.sync.dma_start(xt[1][:], d_xt[P : 2 * P, :])
            ws_load(nc.sync, ws01, 0, kg0, 1)
            zs1 = zpool.tile([P, kg1 * BS], bf16, tag="zs", name="zs1")
            zs_load(nc.sync, zs1, 1, kg1, part=(0, 2))
            zs_load(nc.sync, zs1, 1, kg1, part=(1, 2))
            ztt = cpool.tile([P, 2 * BS], bf16, tag="zt", name="ztt")
            nc.sync.dma_start(ztt[:], d_zt[:])

            consts = {}

            def load_consts():
                bkt, wbtt = [], []
                for kc in range(2):
                    t = cpool.tile([P, OD], bf16, tag=f"bk{kc}", name=f"bk{kc}")
                    nc.sync.dma_start(t[:], d_bk[kc * P : (kc + 1) * P, :])
                    bkt.append(t)
                    if not skip_bias:
                        t = cpool.tile(
                            [P, OD], bf16, tag=f"wbt{kc}", name=f"wbt{kc}"
                        )
                        nc.sync.dma_start(t[:], d_wbt[kc * P : (kc + 1) * P, :])
                        wbtt.append(t)
                consts.update(bkt=bkt, wbtt=wbtt)
                if not skip_bias:
                    bbt = cpool.tile([1, OD], bf16, tag="bb", name="bb")
                    nc.sync.dma_start(bbt[:], d_bb[:])
                    onest = cpool.tile([1, BS], bf16, tag="ones", name="ones")
                    nc.sync.dma_start(onest[:], d_ones[:])
                    consts.update(bbt=bbt, onest=onest)

            # ---- PE warm-up: many short matmuls ramp the HAM clock during
            # the DMA-bound startup; anchored on a memset tile (no DMA dep) ----
            sc_w = cpool.tile([P, 192], bf16, tag="sc_w")
            nc.gpsimd.memset(sc_w[:], 0.0)
            ps_warm = psum.tile([P, 64], f32, tag="ps_warm")

            def warm(n):
                for _ in range(n):
                    nc.tensor.matmul(
                        ps_warm[:], sc_w[:, 0:128], sc_w[:, 128:192],
                        start=True, stop=True,
                    )

            warm(70)

            pso = [
                psum.tile([P, BS], f32, tag=f"ps{oh}", name=f"ps{oh}")
                for oh in range(2)
            ]

            started = [False, False]

            def mm(oh, lhsT, rhs, stop=False, dr=False):
                nc.tensor.matmul(
                    pso[oh][:], lhsT, rhs, start=not started[oh], stop=stop,
                    perf_mode=mybir.MatmulPerfMode.DoubleRow if dr else None,
                )
                started[oh] = True

            # --- main accumulation ---
            for g in range(NG):
                kg = KS[g]
                fp8 = g in FP8_GROUPS
                if g == 0:
                    zs = zs0
                elif g == 1:
                    zs = zs1
                else:
                    zs = zpool.tile([P, kg * BS], bf16, tag="zs")
                    zs_load(nc.sync, zs, g, kg)
                if g == 2:
                    load_consts()
                for ih in range(2):
                    if g == 0 and ih == 0:
                        ws = ws00
                    elif g == 0 and ih == 1:
                        ws = ws01
                    else:
                        if fp8:
                            wdt, wtag = f8e4, "ws8"
                        elif g in E3_GROUPS:
                            wdt, wtag = f8e3, "ws3"
                        else:
                            wdt, wtag = bf16, "ws"
                        ws = wpool.tile([P, kg * OD], wdt, tag=wtag)
                        ws_load(nc.sync, ws, g, kg, ih)
                    # PT[i, (k, b)] = x[i, b] * z[k, b]; x free-broadcast over k
                    pt = ptpool.tile([P, kg * BS], bf16, tag="pt")
                    if g == 0:
                        for q in range(4):
                            kq = kg // 4
                            xb = xt[ih][:].unsqueeze(1).broadcast_to((P, kq, BS))
                            nc.vector.tensor_mul(
                                pt[:, q * kq * BS : (q + 1) * kq * BS].rearrange(
                                    "p (k b) -> p k b", k=kq
                                ),
                                xb,
                                zs[:, q * kq * BS : (q + 1) * kq * BS].rearrange(
                                    "p (k b) -> p k b", k=kq
                                ),
                            )
                    else:
                        xb = xt[ih][:].unsqueeze(1).broadcast_to((P, kg, BS))
                        nc.vector.tensor_mul(
                            pt[:].rearrange("p (k b) -> p k b", k=kg),
                            xb,
                            zs[:].rearrange("p (k b) -> p k b", k=kg),
                        )
                    if fp8:
                        # idle Act engine converts bf16 -> e4m3 (DVE fp8
                        # writes would run at 1x and starve the PE); the x32
                        # scale restores x*z magnitude (x is 1/32-prescaled)
                        pt8 = ptpool.tile([P, kg * BS], f8e4, tag="pt8")
                        QF = kg * BS // 4
                        for qc in range(4):
                            nc.scalar.activation(
                                pt8[:, qc * QF : (qc + 1) * QF],
                                pt[:, qc * QF : (qc + 1) * QF],
                                mybir.ActivationFunctionType.Copy, scale=W_SCALE,
                            )
                        pt = pt8
                    last_grp = (g == NG - 1) and (ih == 1)
                    if fp8:
                        # DoubleRow: 2 k's per matmul; W slab packed as
                        # [P, (kp, oh, j, o')] on the host
                        for kp in range(kg // 2):
                            for oh in range(2):
                                base = (kp * 2 + oh) * 2 * P
                                lhsT = ws[:, base : base + 2 * P].rearrange(
                                    "p (two m) -> p two m", two=2
                                )
                                rhs = pt[
                                    :, 2 * kp * BS : (2 * kp + 2) * BS
                                ].rearrange("p (two b) -> p two b", two=2)
                                mm(oh, lhsT, rhs, dr=True)
                    elif last_grp:
                        # o-half-major so bank 0 finishes early; its drain
                        # overlaps bank 1 tail matmuls.  Drain applies the
                        # 1/32 W-scale compensation; both engines in parallel.
                        for oh in range(2):
                            for kl in range(kg):
                                mm(
                                    oh,
                                    ws[:, kl * OD + oh * P : kl * OD + oh * P + P],
                                    pt[:, kl * BS : (kl + 1) * BS],
                                    stop=(kl == kg - 1),
                                )
                            # output already at final scale: plain parallel
                            # copies (DVE + Act), DMA per half
                            ot = opool.tile(
                                [P, BS], f32, tag=f"ot{oh}", name=f"ot{oh}"
                            )
                            H = BS // 2
                            for h in range(2):
                                hsl = slice(h * H, (h + 1) * H)
                                if h == 0:
                                    nc.vector.tensor_copy(
                                        ot[:, hsl], pso[oh][:, hsl]
                                    )
                                else:
                                    nc.scalar.copy(ot[:, hsl], pso[oh][:, hsl])
                                nc.sync.dma_start(
                                    d_out[oh * P : (oh + 1) * P, hsl], ot[:, hsl]
                                )
                    else:
                        for kl in range(kg):
                            for oh in range(2):
                                mm(
                                    oh,
                                    ws[:, kl * OD + oh * P : kl * OD + oh * P + P],
                                    pt[:, kl * BS : (kl + 1) * BS],
                                )
                            if g == 0 and kl in (1, 3, 5):
                                warm(3)  # fill PE while next PT quarter lands
                if g == 3:
                    # bias matmuls mid-stream: PE warm, consts landed
                    for oh in range(2):
                        osl = slice(oh * P, (oh + 1) * P)
                        mm(oh, consts["bkt"][0][:, osl], ztt[:, 0:BS])
                        mm(oh, consts["bkt"][1][:, osl], ztt[:, BS : 2 * BS])
                        if not skip_bias:
                            mm(oh, consts["wbtt"][0][:, osl], xt[0][:])
                            mm(oh, consts["wbtt"][1][:, osl], xt[1][:])
                            mm(oh, consts["bbt"][:, osl], consts["onest"][:])

    nc.compile()

    _prog_cache[key] = nc
    return nc, "outT"


def _prep_inputs(x, z, W_kernel, W_bias, b_kernel, b_bias):
    x = np.asarray(x, dtype=np.float32)
    z = np.asarray(z, dtype=np.float32)
    W_kernel = np.asarray(W_kernel, dtype=np.float32)
    W_bias = np.asarray(W_bias, dtype=np.float32)
    b_kernel = np.asarray(b_kernel, dtype=np.float32)
    b_bias = np.asarray(b_bias, dtype=np.float32)

    # Wt[(k,i), o] = W_kernel[k, o*XD+i], scaled x32 (drain divides it out)
    Wt = W_kernel.reshape(ZD, OD, XD).transpose(0, 2, 1) * W_SCALE  # [k, i, o]
    W4 = Wt.reshape(ZD, 2, P, OD)  # [k, ih, p, o]
    slabs_b, slabs_8, slabs_3 = [], [], []
    k0 = 0
    for g, kg in enumerate(KS):
        for ih in range(2):
            s = W4[k0 : k0 + kg, ih]  # [kg, P, OD]
            if g in FP8_GROUPS:
                # DoubleRow packing: [P, (kp, oh, j, o')]; UNSCALED (the x32
                # lives in the Act-converted PT for fp8 groups)
                s2 = s.reshape(kg // 2, 2, P, 2, P)  # [kp, j, p, oh, o']
                s2 = s2.transpose(2, 0, 3, 1, 4)     # [p, kp, oh, j, o']
                slabs_8.append(
                    np.clip(np.ascontiguousarray(s2) / W_SCALE, -240, 240)
                    .astype(E4)
                    .reshape(-1)
                )
            elif g in E3_GROUPS:
                sb = np.ascontiguousarray(s.transpose(1, 0, 2))  # [P, kg, OD]
                slabs_3.append(
                    np.clip(sb, -15.5, 15.5).astype(E3).reshape(-1)
                )
            else:
                sb = s.transpose(1, 0, 2)  # [P, kg, OD]
                slabs_b.append(np.ascontiguousarray(sb).astype(BF).reshape(-1))
        k0 += kg
    wtb = np.concatenate(slabs_b)
    wt8 = np.concatenate(slabs_8)
    wt3 = np.concatenate(slabs_3)

    bk = b_kernel.astype(BF)  # [ZD, OD]
    # W_bias matmul consumes the 1/32-scaled x tile: scale wbt up x32
    wbt = np.ascontiguousarray(W_bias.reshape(OD, XD).T * W_SCALE).astype(BF)
    bb = b_bias.reshape(1, OD).astype(BF)
    ones = np.ones((1, BS), dtype=BF)

    in_maps = []
    for c in range(NCORES):
        xs = x[c * BS : (c + 1) * BS]  # [BS, XD]
        zsh = z[c * BS : (c + 1) * BS]  # [BS, ZD]
        xT = np.ascontiguousarray(xs.T / W_SCALE).astype(BF)  # [XD, BS] x 1/32
        zT = np.ascontiguousarray(zsh.T).astype(BF)  # [ZD, BS]
        zt2 = np.concatenate([zT[0:P, :], zT[P : 2 * P, :]], axis=1)
        zslabs = []
        k0 = 0
        for kg in KS:
            row = zT[k0 : k0 + kg].reshape(1, kg * BS)
            zslabs.append(np.broadcast_to(row, (P, kg * BS)).reshape(-1))
            k0 += kg
        zrep = np.ascontiguousarray(np.concatenate(zslabs))
        in_maps.append(
            {
                "xt": xT,
                "zrep": zrep,
                "wtb": wtb,
                "wt8": wt8,
                "wt3": wt3,
                "zt": np.ascontiguousarray(zt2),
                "bk": bk,
                "wbt": wbt,
                "bb": bb,
                "ones": ones,
            }
        )
    return in_maps


def kernel_run(inputs, trace=False, trace_kwargs=None):
    """Run on hardware; returns (out [B,OD] float32, BassKernelResults)."""
    import concourse.bass_utils as bass_utils

    skip_bias = not (
        np.any(np.asarray(inputs["W_bias"])) or np.any(np.asarray(inputs["b_bias"]))
    )
    nc, out_name = _build_program(skip_bias=skip_bias)
    in_maps = _prep_inputs(**inputs)
    res = bass_utils.run_bass_kernel_spmd(
        nc,
        in_maps,
        core_ids=list(range(NCORES)),
        trace=trace,
        **(trace_kwargs or {}),
    )
    out = np.empty((B, OD), dtype=np.float32)
    for c in range(NCORES):
        out[c * BS : (c + 1) * BS, :] = res.results[c][out_name].T
    return out, res


def kernel(x, z, W_kernel, W_bias, b_kernel, b_bias):
    out, _ = kernel_run(
        dict(
            x=x,
            z=z,
            W_kernel=W_kernel,
            W_bias=W_bias,
            b_kernel=b_kernel,
            b_bias=b_bias,
        ),
        trace=False,
    )
    return out


# revision 48
# speedup vs baseline: 1.0240x; 1.0240x over previous
"""Trainium2 Bass kernel for nn_BilinearDense.

Math:
  W = (z @ W_kernel + W_bias).reshape(B, OD, XD)      # per-sample matrix
  b = z @ b_kernel + b_bias                            # per-sample bias
  out[b,o] = sum_i W[b,o,i] x[b,i] + b[b,o]
           = sum_{k,i} z[b,k] x[b,i] W_kernel[k, o*XD+i]  (+ bias terms)

Strategy (8 NeuronCores, batch-sharded, 512 samples/core):
  One long PE accumulation per core:
     outT[o, b] = sum_{ki} Wt[ki, o] * PT[ki, b]
  with Wt[(k,i), o] = W_kernel[k, o*XD+i] and PT[(k,i), b] = z[b,k]*x[b,i]
  built on-chip by the Vector engine (x comes from a 0-stride broadcast AP
  over one resident [XD, BS] tile; z is pre-broadcast in HBM as zrep).

  Mixed precision: 9 of 33 k-groups run as fp8 e4m3 DoubleRow matmuls
  (2 k's contracted per PE pass = 2x FLOP rate, HW-verified); their PT
  is converted bf16 -> e4m3 by the otherwise-idle Activation engine
  (DVE fp8 writes run at 1x and would starve the PE).  Three startup
  groups use e3m4 W (mixed-dtype matmul at full bf16 rate, half DMA).
  Scales: x is pre-divided by 32, bf16/e3m4 W slabs carry x32, fp8 W is
  unscaled with the x32 applied in the Act conversion -- so PSUM holds
  final-scale values and drains with plain copies.  fp8 groups are
  spread every 3rd group: fp8 blocks are locally Act/DVE-bound and must
  interleave with PE-bound bf16 blocks.  Short warm-up matmuls on a
  memset tile ramp the PE clock during the DMA-bound startup.
  Measured rel-err 1.752e-2 (< 2e-2 gate), HW ~221-223us vs 255.6us
  bf16 baseline.
"""

import numpy as np
import ml_dtypes

B, XD, ZD, OD = 4096, 256, 256, 256
NCORES = 8
BS = B // NCORES  # batch shard per core
KG = 8            # k's per group (z-replica slab granularity)
NG = ZD // KG     # 32 groups
P = 128

W_SCALE = 32.0
OUT_DESCALE = 1.0 / W_SCALE
# k-group sizes: two small startup groups (faster pipeline fill), then 8s
KS = [4, 4] + [8] * 31
NG = len(KS)
FP8_GROUPS = (5, 8, 11, 14, 17, 20, 23, 26, 29)  # e4m3 DoubleRow groups
E3_GROUPS = (0, 1, 2)                        # e3m4 W (halves startup W DMA)

BF = ml_dtypes.bfloat16
E4 = ml_dtypes.float8_e4m3
E3 = ml_dtypes.float8_e3m4

_prog_cache = {}


def _build_program(skip_bias=False):
    key = ("nc", skip_bias)
    if key in _prog_cache:
        return _prog_cache[key], "outT"

    import concourse.bass as bass
    import concourse.tile as tile
    from concourse import bacc, mybir

    bf16 = mybir.dt.bfloat16
    f8e4 = mybir.dt.float8e4
    f8e3 = mybir.dt.float8e3
    f32 = mybir.dt.float32

    nc = bacc.Bacc(
        "TRN2", target_bir_lowering=False, debug=False, num_devices=NCORES
    )

    kb = sum(kg for g, kg in enumerate(KS)
             if g not in FP8_GROUPS and g not in E3_GROUPS)
    k8 = sum(KS[g] for g in FP8_GROUPS)
    k3 = sum(KS[g] for g in E3_GROUPS)
    d_xt = nc.dram_tensor("xt", [XD, BS], bf16, kind="ExternalInput").ap()
    d_zrep = nc.dram_tensor("zrep", [ZD * P * BS], bf16, kind="ExternalInput").ap()
    d_wtb = nc.dram_tensor("wtb", [kb * 2 * P * OD], bf16, kind="ExternalInput").ap()
    d_wt8 = nc.dram_tensor("wt8", [k8 * 2 * P * OD], f8e4, kind="ExternalInput").ap()
    d_wt3 = nc.dram_tensor("wt3", [k3 * 2 * P * OD], f8e3, kind="ExternalInput").ap()
    d_zt = nc.dram_tensor("zt", [P, 2 * BS], bf16, kind="ExternalInput").ap()
    d_bk = nc.dram_tensor("bk", [ZD, OD], bf16, kind="ExternalInput").ap()
    d_wbt = nc.dram_tensor("wbt", [XD, OD], bf16, kind="ExternalInput").ap()
    d_bb = nc.dram_tensor("bb", [1, OD], bf16, kind="ExternalInput").ap()
    d_ones = nc.dram_tensor("ones", [1, BS], bf16, kind="ExternalInput").ap()
    d_out = nc.dram_tensor("outT", [OD, BS], f32, kind="ExternalOutput").ap()

    # slab offsets (separate streams per W dtype)
    wt_off = {}
    zr_off = {}
    offb = off8 = off3 = zoff = 0
    for _g, _kg in enumerate(KS):
        zr_off[_g] = zoff
        zoff += P * _kg * BS
        for _ih in range(2):
            if _g in FP8_GROUPS:
                wt_off[(_g, _ih)] = off8
                off8 += P * _kg * OD
            elif _g in E3_GROUPS:
                wt_off[(_g, _ih)] = off3
                off3 += P * _kg * OD
            else:
                wt_off[(_g, _ih)] = offb
                offb += P * _kg * OD

    def zs_load(engine, zs, g, kg, part=None):
        o = zr_off[g]
        src = d_zrep[o : o + P * kg * BS].rearrange("(p f) -> p f", p=P)
        if part is None:
            engine.dma_start(zs[:], src)
        else:
            i, n = part
            F = kg * BS // n
            engine.dma_start(zs[:, i * F : (i + 1) * F], src[:, i * F : (i + 1) * F])

    def ws_load(engine, ws, g, kg, ih, part=None):
        o = wt_off[(g, ih)]
        d = (
            d_wt8
            if g in FP8_GROUPS
            else (d_wt3 if g in E3_GROUPS else d_wtb)
        )
        src = d[o : o + P * kg * OD].rearrange("(p f) -> p f", p=P)
        if part is None:
            engine.dma_start(ws[:], src)
        else:
            i, n = part
            F = kg * OD // n
            engine.dma_start(ws[:, i * F : (i + 1) * F], src[:, i * F : (i + 1) * F])

    with tile.TileContext(nc) as tc:
        with (
            tc.tile_pool(name="const", bufs=1) as cpool,
            tc.tile_pool(name="zslab", bufs=5) as zpool,
            tc.tile_pool(name="wslab", bufs=5) as wpool,
            tc.tile_pool(name="pt", bufs=4) as ptpool,
            tc.tile_pool(name="outp", bufs=1) as opool,
            tc.tile_pool(name="psum", bufs=1, space="PSUM") as psum,
        ):
            # ---- initial loads, most-critical first ----
            xt = [
                cpool.tile([P, BS], bf16, tag=f"xt{ih}", name=f"xt{ih}")
                for ih in range(2)
            ]
            kg0, kg1 = KS[0], KS[1]
            zs0 = zpool.tile([P, kg0 * BS], bf16, tag="zs", name="zs0")
            ws00 = wpool.tile([P, kg0 * OD], f8e3, tag="ws3", name="ws00")
            ws01 = wpool.tile([P, kg0 * OD], f8e3, tag="ws3", name="ws01")

            nc.sync.dma_start(xt[0][:], d_xt[0:P, :])
            zs_load(nc.sync, zs0, 0, kg0, part=(0, 2))
            ws_load(nc.sync, ws00, 0, kg0, 0)
            zs_load(nc.sync, zs0, 0, kg0, part=(1, 2))
            nc.sync.dma_start(xt[1][:], d_xt[P : 2 * P, :])
            ws_load(nc.sync, ws01, 0, kg0, 1)
            zs1 = zpool.tile([P, kg1 * BS], bf16, tag="zs", name="zs1")
            zs_load(nc.sync, zs1, 1, kg1, part=(0, 2))
            zs_load(nc.sync, zs1, 1, kg1, part=(1, 2))
            ztt = cpool.tile([P, 2 * BS], bf16, tag="zt", name="ztt")
            nc.sync.dma_start(ztt[:], d_zt[:])

            consts = {}

            def load_consts():
                bkt, wbtt = [], []
                for kc in range(2):
                    t = cpool.tile([P, OD], bf16, tag=f"bk{kc}", name=f"bk{kc}")
                    nc.sync.dma_start(t[:], d_bk[kc * P : (kc + 1) * P, :])
                    bkt.append(t)
                    if not skip_bias:
                        t = cpool.tile(
                            [P, OD], bf16, tag=f"wbt{kc}", name=f"wbt{kc}"
                        )
                        nc.sync.dma_start(t[:], d_wbt[kc * P : (kc + 1) * P, :])
                        wbtt.append(t)
                consts.update(bkt=bkt, wbtt=wbtt)
                if not skip_bias:
                    bbt = cpool.tile([1, OD], bf16, tag="bb", name="bb")
                    nc.sync.dma_start(bbt[:], d_bb[:])
                    onest = cpool.tile([1, BS], bf16, tag="ones", name="ones")
                    nc.sync.dma_start(onest[:], d_ones[:])
                    consts.update(bbt=bbt, onest=onest)

            # ---- PE warm-up: many short matmuls ramp the HAM clock during
            # the DMA-bound startup; anchored on a memset tile (no DMA dep) ----
            sc_w = cpool.tile([P, 192], bf16, tag="sc_w")
            nc.gpsimd.memset(sc_w[:], 0.0)
            ps_warm = psum.tile([P, 64], f32, tag="ps_warm")

            def warm(n):
                for _ in range(n):
                    nc.tensor.matmul(
                        ps_warm[:], sc_w[:, 0:128], sc_w[:, 128:192],
                        start=True, stop=True,
                    )

            warm(70)

            pso = [
                psum.tile([P, BS], f32, tag=f"ps{oh}", name=f"ps{oh}")
                for oh in range(2)
            ]

            started = [False, False]

            def mm(oh, lhsT, rhs, stop=False, dr=False):
                nc.tensor.matmul(
                    pso[oh][:], lhsT, rhs, start=not started[oh], stop=stop,
                    perf_mode=mybir.MatmulPerfMode.DoubleRow if dr else None,
                )
                started[oh] = True

            # --- main accumulation ---
            for g in range(NG):
                kg = KS[g]
                fp8 = g in FP8_GROUPS
                if g == 0:
                    zs = zs0
                elif g == 1:
                    zs = zs1
                else:
                    zs = zpool.tile([P, kg * BS], bf16, tag="zs")
                    zs_load(nc.sync, zs, g, kg)
                if g == 2:
                    load_consts()
                for ih in range(2):
                    if g == 0 and ih == 0:
                        ws = ws00
                    elif g == 0 and ih == 1:
                        ws = ws01
                    else:
                        if fp8:
                            wdt, wtag = f8e4, "ws8"
                        elif g in E3_GROUPS:
                            wdt, wtag = f8e3, "ws3"
                        else:
                            wdt, wtag = bf16, "ws"
                        ws = wpool.tile([P, kg * OD], wdt, tag=wtag)
                        ws_load(nc.sync, ws, g, kg, ih)
                    # PT[i, (k, b)] = x[i, b] * z[k, b]; x free-broadcast over k
                    pt = ptpool.tile([P, kg * BS], bf16, tag="pt")
                    if g == 0:
                        for q in range(4):
                            kq = kg // 4
                            xb = xt[ih][:].unsqueeze(1).broadcast_to((P, kq, BS))
                            nc.vector.tensor_mul(
                                pt[:, q * kq * BS : (q + 1) * kq * BS].rearrange(
                                    "p (k b) -> p k b", k=kq
                                ),
                                xb,
                                zs[:, q * kq * BS : (q + 1) * kq * BS].rearrange(
                                    "p (k b) -> p k b", k=kq
                                ),
                            )
                    else:
                        xb = xt[ih][:].unsqueeze(1).broadcast_to((P, kg, BS))
                        nc.vector.tensor_mul(
                            pt[:].rearrange("p (k b) -> p k b", k=kg),
                            xb,
                            zs[:].rearrange("p (k b) -> p k b", k=kg),
                        )
                    if fp8:
                        # idle Act engine converts bf16 -> e4m3 (DVE fp8
                        # writes would run at 1x and starve the PE); the x32
                        # scale restores x*z magnitude (x is 1/32-prescaled)
                        pt8 = ptpool.tile([P, kg * BS], f8e4, tag="pt8")
                        HF = kg * BS // 2
                        nc.scalar.activation(
                            pt8[:, 0:HF], pt[:, 0:HF],
                            mybir.ActivationFunctionType.Copy, scale=W_SCALE,
                        )
                        nc.scalar.activation(
                            pt8[:, HF:], pt[:, HF:],
                            mybir.ActivationFunctionType.Copy, scale=W_SCALE,
                        )
                        pt = pt8
                    last_grp = (g == NG - 1) and (ih == 1)
                    if fp8:
                        # DoubleRow: 2 k's per matmul; W slab packed as
                        # [P, (kp, oh, j, o')] on the host
                        for kp in range(kg // 2):
                            for oh in range(2):
                                base = (kp * 2 + oh) * 2 * P
                                lhsT = ws[:, base : base + 2 * P].rearrange(
                                    "p (two m) -> p two m", two=2
                                )
                                rhs = pt[
                                    :, 2 * kp * BS : (2 * kp + 2) * BS
                                ].rearrange("p (two b) -> p two b", two=2)
                                mm(oh, lhsT, rhs, dr=True)
                    elif last_grp:
                        # o-half-major so bank 0 finishes early; its drain
                        # overlaps bank 1 tail matmuls.  Drain applies the
                        # 1/32 W-scale compensation; both engines in parallel.
                        for oh in range(2):
                            for kl in range(kg):
                                mm(
                                    oh,
                                    ws[:, kl * OD + oh * P : kl * OD + oh * P + P],
                                    pt[:, kl * BS : (kl + 1) * BS],
                                    stop=(kl == kg - 1),
                                )
                            # output already at final scale: plain parallel
                            # copies (DVE + Act), DMA per half
                            ot = opool.tile(
                                [P, BS], f32, tag=f"ot{oh}", name=f"ot{oh}"
                            )
                            H = BS // 2
                            for h in range(2):
                                hsl = slice(h * H, (h + 1) * H)
                                if h == 0:
                                    nc.vector.tensor_copy(
                                        ot[:, hsl], pso[oh][:, hsl]
                                    )
                                else:
                                    nc.scalar.copy(ot[:, hsl], pso[oh][:, hsl])
                                nc.sync.dma_start(
                                    d_out[oh * P : (oh + 1) * P, hsl], ot[:, hsl]
                                )
                    else:
                        for kl in range(kg):
                            for oh in range(2):
                                mm(
                                    oh,
                                    ws[:, kl * OD + oh * P : kl * OD + oh * P + P],
                                    pt[:, kl * BS : (kl + 1) * BS],
                                )
                            if g == 0 and kl in (1, 3, 5):
                                warm(3)  # fill PE while next PT quarter lands
                if g == 3:
                    # bias matmuls mid-stream: PE warm, consts landed
                    for oh in range(2):
                        osl = slice(oh * P, (oh + 1) * P)
                        mm(oh, consts["bkt"][0][:, osl], ztt[:, 0:BS])
                        mm(oh, consts["bkt"][1][:, osl], ztt[:, BS : 2 * BS])
                        if not skip_bias:
                            mm(oh, consts["wbtt"][0][:, osl], xt[0][:])
                            mm(oh, consts["wbtt"][1][:, osl], xt[1][:])
                            mm(oh, consts["bbt"][:, osl], consts["onest"][:])

    nc.compile()

    _prog_cache[key] = nc
    return nc, "outT"


def _prep_inputs(x, z, W_kernel, W_bias, b_kernel, b_bias):
    x = np.asarray(x, dtype=np.float32)
    z = np.asarray(z, dtype=np.float32)
    W_kernel = np.asarray(W_kernel, dtype=np.float32)
    W_bias = np.asarray(W_bias, dtype=np.float32)
    b_kernel = np.asarray(b_kernel, dtype=np.float32)
    b_bias = np.asarray(b_bias, dtype=np.float32)

    # Wt[(k,i), o] = W_kernel[k, o*XD+i], scaled x32 (drain divides it out)
    Wt = W_kernel.reshape(ZD, OD, XD).transpose(0, 2, 1) * W_SCALE  # [k, i, o]
    W4 = Wt.reshape(ZD, 2, P, OD)  # [k, ih, p, o]
    slabs_b, slabs_8, slabs_3 = [], [], []
    k0 = 0
    for g, kg in enumerate(KS):
        for ih in range(2):
            s = W4[k0 : k0 + kg, ih]  # [kg, P, OD]
            if g in FP8_GROUPS:
                # DoubleRow packing: [P, (kp, oh, j, o')]; UNSCALED (the x32
                # lives in the Act-converted PT for fp8 groups)
                s2 = s.reshape(kg // 2, 2, P, 2, P)  # [kp, j, p, oh, o']
                s2 = s2.transpose(2, 0, 3, 1, 4)     # [p, kp, oh, j, o']
                slabs_8.append(
                    np.clip(np.ascontiguousarray(s2) / W_SCALE, -240, 240)
                    .astype(E4)
                    .reshape(-1)
                )
            elif g in E3_GROUPS:
                sb = np.ascontiguousarray(s.transpose(1, 0, 2))  # [P, kg, OD]
                slabs_3.append(
                    np.clip(sb, -15.5, 15.5).astype(E3).reshape(-1)
                )
            else:
                sb = s.transpose(1, 0, 2)  # [P, kg, OD]
                slabs_b.append(np.ascontiguousarray(sb).astype(BF).reshape(-1))
        k0 += kg
    wtb = np.concatenate(slabs_b)
    wt8 = np.concatenate(slabs_8)
    wt3 = np.concatenate(slabs_3)

    bk = b_kernel.astype(BF)  # [ZD, OD]
    # W_bias matmul consumes the 1/32-scaled x tile: scale wbt up x32
    wbt = np.ascontiguousarray(W_bias.reshape(OD, XD).T * W_SCALE).astype(BF)
    bb = b_bias.reshape(1, OD).astype(BF)
    ones = np.ones((1, BS), dtype=BF)

    in_maps = []
    for c in range(NCORES):
        xs = x[c * BS : (c + 1) * BS]  # [BS, XD]
        zsh = z[c * BS : (c + 1) * BS]  # [BS, ZD]
        xT = np.ascontiguousarray(xs.T / W_SCALE).astype(BF)  # [XD, BS] x 1/32
        zT = np.ascontiguousarray(zsh.T).astype(BF)  # [ZD, BS]
        zt2 = np.concatenate([zT[0:P, :], zT[P : 2 * P, :]], axis=1)
        zslabs = []
        k0 = 0
        for kg in KS:
            row = zT[k0 : k0 + kg].reshape(1, kg * BS)
            zslabs.append(np.broadcast_to(row, (P, kg * BS)).reshape(-1))
            k0 += kg
        zrep = np.ascontiguousarray(np.concatenate(zslabs))
        in_maps.append(
            {
                "xt": xT,
                "zrep": zrep,
                "wtb": wtb,
                "wt8": wt8,
                "wt3": wt3,
                "zt": np.ascontiguousarray(zt2),
                "bk": bk,
                "wbt": wbt,
                "bb": bb,
                "ones": ones,
            }
        )
    return in_maps


def kernel_run(inputs, trace=False, trace_kwargs=None):
    """Run on hardware; returns (out [B,OD] float32, BassKernelResults)."""
    import concourse.bass_utils as bass_utils

    skip_bias = not (
        np.any(np.asarray(inputs["W_bias"])) or np.any(np.asarray(inputs["b_bias"]))
    )
    nc, out_name = _build_program(skip_bias=skip_bias)
    in_maps = _prep_inputs(**inputs)
    res = bass_utils.run_bass_kernel_spmd(
        nc,
        in_maps,
        core_ids=list(range(NCORES)),
        trace=trace,
        **(trace_kwargs or {}),
    )
    out = np.empty((B, OD), dtype=np.float32)
    for c in range(NCORES):
        out[c * BS : (c + 1) * BS, :] = res.results[c][out_name].T
    return out, res


def kernel(x, z, W_kernel, W_bias, b_kernel, b_bias):
    out, _ = kernel_run(
        dict(
            x=x,
            z=z,
            W_kernel=W_kernel,
            W_bias=W_bias,
            b_kernel=b_kernel,
            b_bias=b_bias,
        ),
        trace=False,
    )
    return out
